# revision 1
# baseline (speedup 1.0000x reference)
# Trainium2 Bass kernel for nn_CapsLayer_63934883168634.
#
# Math: the reference's routing softmax is over a size-1 axis, so the
# coupling coefficients are identically 1.0 and the 3-iteration routing
# loop is a fixed point.  The whole module reduces to
#     s[b, j, l] = sum_{i,k} inputs[b, i, k] * W[i, j, k, l]
#     vj         = squash(s, over l)
# i.e. one matmul [B, I*K] @ [I*K, J*L] = [64,16384]@[16384,512] plus a
# tiny per-(b, j) squash over L=16.
#
# Sharding: over J (num_caps).  Each of the 8 cores computes 4 output
# capsules: a [64, 16384] @ [16384, 64] matmul + squash.  Per-core HBM
# traffic = full inputs (4 MiB) + W shard (4 MiB).  Inputs/W are
# pre-swizzled on the host so each SBUF tile loads with a fully
# contiguous per-partition DMA.

import numpy as np

B, I, K, J, L = 64, 2048, 8, 32, 16
IK = I * K              # contraction length = 16384
N_CORES = 8
JPC = J // N_CORES      # 4 capsules per core
M = B                   # matmul M (output partitions) = 64
N = JPC * L             # matmul N (free) = 64
P = 128                 # contraction chunk = PE partition dim
NCH = IK // P           # 128 accumulating matmuls

_session = None


def _build_session():
    """Build + compile the Bass module once per process."""
    from contextlib import ExitStack

    import concourse.bacc as bacc
    import concourse.mybir as mybir
    import concourse.tile as tile

    f32 = mybir.dt.float32

    nc = bacc.Bacc(
        "TRN2",
        target_bir_lowering=False,
        debug=False,
        enable_asserts=False,
        num_devices=N_CORES,
    )
    # Host pre-swizzled layouts ([P, NCH * free]): column block c holds
    # contraction rows [c*128, (c+1)*128) for all 64 free elements.
    a_d = nc.dram_tensor("a", [P, NCH * M], f32, kind="ExternalInput").ap()
    w_d = nc.dram_tensor("w", [P, NCH * N], f32, kind="ExternalInput").ap()
    o_d = nc.dram_tensor("o", [M, N], f32, kind="ExternalOutput").ap()

    with tile.TileContext(nc) as tc, ExitStack() as ctx:
        apool = ctx.enter_context(tc.tile_pool(name="apool", bufs=1))
        wpool = ctx.enter_context(tc.tile_pool(name="wpool", bufs=1))
        spool = ctx.enter_context(tc.tile_pool(name="spool", bufs=1))
        ppool = ctx.enter_context(tc.tile_pool(name="ppool", bufs=1, space="PSUM"))

        # epsilon bias for sqrt(s2 + 1e-7)
        eps = spool.tile([128, 1], f32, name="eps")
        nc.vector.memset(eps[:, :], 1e-7)

        # Graded DMA chunking (in units of 64-elem contraction groups):
        # small first chunk so the first matmuls start early, ~1 MiB middles
        # for DMA efficiency.  a-chunks go on the SP HWDGE ring (nc.sync),
        # w-chunks on the ACT ring (nc.scalar) so each a/w pair streams
        # concurrently.
        grades = [16, 24, 32, 32, 24]
        assert sum(grades) == NCH
        a_tiles, w_tiles = [], []
        off0 = 0
        for g, ng in enumerate(grades):
            csz = ng * M
            at = apool.tile([P, csz], f32, name=f"at{g}", tag=f"at{g}")
            nc.sync.dma_start(out=at[:, :], in_=a_d[:, off0 * M:(off0 + ng) * M])
            wt = wpool.tile([P, csz], f32, name=f"wt{g}", tag=f"wt{g}")
            nc.scalar.dma_start(out=wt[:, :], in_=w_d[:, off0 * M:(off0 + ng) * M])
            a_tiles.append((at, ng))
            w_tiles.append((wt, ng))
            off0 += ng
            if g == 1:
                # ACT-table warmup for Square/Sqrt, emitted AFTER the first
                # two w-chunk DMA issues: the table loads ride the same ACT
                # HWDGE ring as the w-chunks, so issuing them here keeps
                # w0/w1 (which gate the first matmuls) ahead of the table
                # DMAs while still loading the tables long before the squash
                # needs them.
                warm = spool.tile([128, 1], f32, name="warm")
                nc.scalar.square(warm[:, :], eps[:, :])
                nc.scalar.activation(
                    warm[:, :], eps[:, :], mybir.ActivationFunctionType.Sqrt)

        # s[b, jl] accumulated over 128 chunks of the contraction, in chunk
        # order so each group's matmuls wait only on its own pair of DMAs.
        # M=64 only fills half the PE array's columns, so even chunks run at
        # tile_position (0,0) and odd chunks concurrently at (0,64) into the
        # upper PSUM partitions (two accumulators, summed afterwards).
        ps = ppool.tile([2 * M, N], f32, name="ps")
        c = 0
        for g, ng in enumerate(grades):
            at = a_tiles[g][0]
            wt = w_tiles[g][0]
            for off in range(ng):
                sl = slice(off * M, off * M + M)
                half = c % 2
                nc.tensor.matmul(
                    ps[half * M:(half + 1) * M, :],
                    lhsT=at[:, sl],
                    rhs=wt[:, sl],
                    start=(c < 2),
                    stop=(c >= NCH - 2),
                    tile_position=(0, half * M),
                )
                c += 1

        cp = spool.tile([M, N], f32, name="cp")
        nc.vector.tensor_copy(cp[:, :], ps[M:2 * M, :])
        s_sb = spool.tile([M, N], f32, name="s_sb")
        nc.vector.tensor_add(s_sb[:, :], ps[:M, :], cp[:, :])

        # squash over l within each of the 4 capsules:
        #   out = s * s2 / ((1 + s2) * sqrt(s2 + 1e-7)),  s2 = sum_l s^2
        # Square+reduce fused via ACTIVATE accum_out (sum over free dim),
        # one slice per capsule, all on the scalar engine so the sqrt that
        # follows needs no cross-engine hop.
        sq = spool.tile([M, N], f32, name="sq")
        s2 = spool.tile([M, JPC], f32, name="s2")
        for j in range(JPC):
            nc.scalar.activation(
                sq[:, j * L:(j + 1) * L],
                s_sb[:, j * L:(j + 1) * L],
                mybir.ActivationFunctionType.Square,
                accum_out=s2[:, j:j + 1],
            )
        rt = spool.tile([M, JPC], f32, name="rt")
        nc.scalar.activation(
            rt[:, :], s2[:, :], mybir.ActivationFunctionType.Sqrt,
            bias=eps[:M, :],
        )
        den = spool.tile([M, JPC], f32, name="den")
        nc.vector.scalar_tensor_tensor(
            den[:, :], s2[:, :], 1.0, rt[:, :],
            op0=mybir.AluOpType.add, op1=mybir.AluOpType.mult,
        )
        rec = spool.tile([M, JPC], f32, name="rec")
        nc.vector.reciprocal(rec[:, :], den[:, :])
        f = spool.tile([M, JPC], f32, name="f")
        nc.vector.tensor_mul(f[:, :], s2[:, :], rec[:, :])

        from concourse.bass import broadcast_tensor_aps

        out_t = spool.tile([M, N], f32, name="out_t")
        s3 = s_sb[:, :].rearrange("p (j l) -> p j l", l=L)
        f3 = f[:, :].rearrange("p (j l) -> p j l", l=1)
        s3b, f3b = broadcast_tensor_aps(s3, f3)
        nc.vector.tensor_mul(
            out_t[:, :].rearrange("p (j l) -> p j l", l=L), s3b, f3b
        )

        # output split across both HWDGE rings so the two ~8 KB halves'
        # completion receipts overlap
        nc.sync.dma_start(out=o_d[:, :N // 2], in_=out_t[:, :N // 2])
        nc.scalar.dma_start(out=o_d[:, N // 2:], in_=out_t[:, N // 2:])

    nc.compile()
    return nc


def _swizzle(mat):
    """[IK, F] f32 -> [128, NCH*F] where col block c = rows [c*128,(c+1)*128)."""
    f = mat.shape[1]
    return np.ascontiguousarray(
        mat.reshape(NCH, P, f).transpose(1, 0, 2).reshape(P, NCH * f)
    )


def _make_in_maps(inputs):
    x = np.ascontiguousarray(np.asarray(inputs["inputs"], dtype=np.float32))
    W = np.ascontiguousarray(np.asarray(inputs["W"], dtype=np.float32))

    # a[ik, b] = x[b, i, k]
    a_sw = _swizzle(x.reshape(B, IK).T)
    in_maps = []
    for c in range(N_CORES):
        # wf[ik, j_local*L + l] = W[i, 4c + j_local, k, l]
        wc = W[:, c * JPC:(c + 1) * JPC, :, :]          # [I, JPC, K, L]
        wf = wc.transpose(0, 2, 1, 3).reshape(IK, JPC * L)
        in_maps.append({"a": a_sw, "w": _swizzle(wf)})
    return in_maps


def kernel(**inputs):
    global _session
    from concourse.bass_utils import run_bass_kernel_spmd

    if _session is None:
        _session = _build_session()

    in_maps = _make_in_maps(inputs)
    try:
        res = run_bass_kernel_spmd(_session, in_maps, list(range(N_CORES)))
    except Exception:
        # the shared device occasionally reports a transient
        # NRT_EXEC_UNIT_UNRECOVERABLE; one retry clears it
        res = run_bass_kernel_spmd(_session, in_maps, list(range(N_CORES)))

    # gather: core c's [64, 64] block covers capsules j in [4c, 4c+4)
    parts = [res.results[c]["o"].reshape(B, JPC, L) for c in range(N_CORES)]
    vj = np.concatenate(parts, axis=1).reshape(B, 1, J, L, 1)
    return np.ascontiguousarray(vj.astype(np.float32))

